# revision 48
# baseline (speedup 1.0000x reference)
"""Trainium2 Bass kernel for nn_Cross_Attention (linear attention + 1x1 conv + LayerNorm).

Math (per batch b):
  kq = x2[b].T (channels-first), heads h=8, 64 ch/head
  keys    = softmax(kq) over tokens N
  queries = softmax(kq) over channels-within-head
  context[h] = keys[h] @ v[h].T          (v = x1[b].T)       [64, 64]
  attended[h] = context[h].T @ queries[h]                    [64, N]
  reproj = conv_w @ concat(attended) + conv_b                [1024, N]
  out = LayerNorm_channels(reproj.T)                         [N, 1024]

Key restructure vs the straightforward pipeline: reproj = cw @ BD(ctx) @ q
is associatively refolded as (cw @ BD(ctx)) @ q, so the big GEMM consumes
the (transposed, query-normalized) exp(kq) directly and the attended
intermediate never materializes.  Mt = BD(ctx)^T @ cw^T is a tiny [512,1024]
precompute per core.

Query-softmax normalization (per head+token scalar) is applied in channel-
major layout: 1/qz head-pair rows are broadcast across partitions by a tiny
K=2 selector matmul into PSUM and fused into one elementwise multiply per
512-token block, instead of 128 tiny per-head tensor_scalars.

LayerNorm: bn_stats/bn_aggr on DVE, then the normalize itself runs on the
Scalar engine as Identity(x * rstd + (-mu*rstd)) with per-partition scale /
bias operands, reading conv PSUM directly and emitting bf16.

Sharding: 8 cores = 4 batches x 2 token-halves. Each core receives the full
batch (needed for the token-axis softmax + context), computes context
redundantly within the pair, and produces its own 2048-token half of the
output (bf16, upcast on host). No cross-core communication.

Numerics: exp/softmax inputs are bounded (randn), so the max-subtraction is
skipped. Matmuls run in bf16 with fp32 PSUM accumulation. The softmax-over-N
denominator is obtained by a ones-column matmul fused into the context
accumulation. The conv bias is injected into PSUM via a K=1 ones-row matmul
so LayerNorm stats can be taken directly from PSUM.
"""

import numpy as np
import ml_dtypes
from contextlib import ExitStack

import concourse.bass as bass
import concourse.bacc as bacc
import concourse.tile as tile
from concourse import mybir
from concourse.bass_utils import run_bass_kernel_spmd
from concourse.masks import make_identity

BF16 = mybir.dt.bfloat16
F32 = mybir.dt.float32
NPBF16 = ml_dtypes.bfloat16

P = 128          # partitions
NQ = 2048        # tokens owned by this core (query half)
NF = 4096        # full token count per batch
D = 512          # input channels
H = 8            # heads
HC = 64          # channels per head
O = 1024         # conv output channels
TQ = NQ // P     # 16 query-half token tiles
TF = NF // P     # 32 full token tiles
NCH = D // P     # 4 channel chunks (2 heads each)
LN_EPS = 1e-5
B = 4
N_CORES = 8
GRP = 4          # token tiles per input DMA

Exp = mybir.ActivationFunctionType.Exp
Sqrt = mybir.ActivationFunctionType.Sqrt
Ident = mybir.ActivationFunctionType.Identity
Mult = mybir.AluOpType.mult


def _build_program():
    # Bacc (not plain Bass): its finalize() runs move_matmul_waits_to_
    # ldweights + generate_event_semaphores, which split multi-wait
    # instructions into EventSemaphore preludes — the HW encodings allow
    # at most 1 inline wait (2 for EventSemaphore).
    nc = bacc.Bacc()
    # x1 halves arrive pre-interleaved as [NQ, 4, 129]: four 128-channel
    # chunks each followed by a literal 1.0 column (softmax-Z ones fused
    # into the context matmul's moving operand).
    x1a = nc.declare_dram_parameter("x1a", [NQ, D + NCH], BF16, isOutput=False)
    x1b = nc.declare_dram_parameter("x1b", [NQ, D + NCH], BF16, isOutput=False)
    x2a = nc.declare_dram_parameter("x2a", [NQ, D], BF16, isOutput=False)
    x2b = nc.declare_dram_parameter("x2b", [NQ, D], BF16, isOutput=False)
    cwt = nc.declare_dram_parameter("cwt", [D, O], BF16, isOutput=False)
    cbp = nc.declare_dram_parameter("cb", [1, O], BF16, isOutput=False)
    selp = nc.declare_dram_parameter("sel", [2, P], BF16, isOutput=False)
    out = nc.declare_dram_parameter("out", [NQ, O], BF16, isOutput=True)

    with tile.TileContext(nc) as tc, ExitStack() as ctx:
        singles = ctx.enter_context(tc.tile_pool(name="singles", bufs=1))
        # DMA-written pools get one buf per tile (no slot reuse): a reused
        # slot's DMA needs WAR + 2-queue WAW waits = 3 > the 2-wait limit of
        # the DMA descriptor encoding. Fresh slots -> input DMAs wait-free.
        kqpool = ctx.enter_context(tc.tile_pool(name="kq", bufs=TF // GRP))
        vpool = ctx.enter_context(tc.tile_pool(name="v", bufs=TF // GRP))
        ekqpool = ctx.enter_context(tc.tile_pool(name="ekq", bufs=6))
        bdpool = ctx.enter_context(tc.tile_pool(name="bd", bufs=NCH))
        bdTpool = ctx.enter_context(tc.tile_pool(name="bdT", bufs=NCH))
        lnpool = ctx.enter_context(tc.tile_pool(name="ln", bufs=6))
        outpool = ctx.enter_context(tc.tile_pool(name="outp", bufs=3))
        miscpool = ctx.enter_context(tc.tile_pool(name="misc", bufs=8))
        rqsbpool = ctx.enter_context(tc.tile_pool(name="rqsb", bufs=4))
        # PSUM: 8 banks of 2KB/partition total.
        # a: ctx accumulators (phase 1, 4 live) then conv halves (phase 2,
        #    disjoint lifetime) -> 4 banks.
        # b: ekq transposes (ph1) / Mt halves (ph1.5) -> 2 banks.
        # c: rqz transposes (ph1) / bdT transposes (ph1.5) -> 2 banks.
        ps_a = ctx.enter_context(tc.tile_pool(name="ps_a", bufs=4, space="PSUM"))
        ps_b = ctx.enter_context(tc.tile_pool(name="ps_b", bufs=2, space="PSUM"))
        ps_c = ctx.enter_context(tc.tile_pool(name="ps_c", bufs=2, space="PSUM"))

        # constants (weight/bias/selector loads are emitted AFTER the input
        # loop so the sync queue issues the latency-critical kq loads first)
        ident = singles.tile([P, P], BF16)
        make_identity(nc, ident)
        ones_row = singles.tile([1, P], BF16)
        nc.vector.memset(ones_row, 1.0)
        eps_t = singles.tile([P, 1], F32)
        nc.vector.memset(eps_t, LN_EPS)
        cw_sb = singles.tile([P, NCH, O], BF16)
        cb_sb = singles.tile([1, O], BF16)

        # big SBUF intermediates
        rqzT = singles.tile([2, NCH, NQ], BF16)   # 1/qz, head-pair-major
        qtraw = singles.tile([P, NCH, NQ], BF16)  # exp(kq)^T, channel-major
        qts = singles.tile([P, NCH, NQ], BF16)    # normalized queries^T
        mt_sb = singles.tile([P, NCH, O], BF16)   # Mt = BD(ctx)^T @ cw^T
        # head-pair selector (host constant): sel^T @ rqzT[:, c] broadcasts
        # head 2c to partitions 0..63 and head 2c+1 to 64..127 (PSUM),
        # replacing the per-head scalar normalization with one fused multiply.
        sel = singles.tile([2, P], BF16)
        # tiny constants via GpSimd SWDGE: keeps both HWDGE queues free for
        # the latency-critical input stream.
        nc.gpsimd.dma_start(sel, selp[:, :])
        # block-diagonal context tiles, zeroed up front (off-phase memsets)
        bds = []
        for c in range(NCH):
            bd = bdpool.tile([P, P], BF16, tag="bd", name=f"bd{c}")
            nc.vector.memset(bd, 0.0)
            bds.append(bd)

        # ---- Phase 1: exp(kq); context/Z accumulation over all 32 tiles;
        # for own-half tiles also transpose exp(kq) (sharing the stationary
        # operand with the context matmul) and produce 1/qz rows.
        # Input loads are batched 4 token-tiles per DMA: descriptor issue on
        # the sync queue costs ~620ns per DMA, so 64 single-tile loads would
        # pace the whole phase.
        ctx_ps = [ps_a.tile([P, P + 1], F32, tag="a", name=f"ctxps{i}")
                  for i in range(NCH)]
        BLK = NQ // 4

        def q_normalize(c, blk):
            # one (chunk, 512-token block) of query normalization:
            # PE broadcasts 1/qz per head pair into PSUM, DVE casts it to
            # SBUF, and the multiply itself runs on the otherwise-idle
            # GpSimd (SBUF-only operands), keeping DVE clear for the
            # context-fold chain.
            bsl = slice(blk * BLK, (blk + 1) * BLK)
            rqm = (ps_c if c % 2 == 0 else ps_b).tile(
                [P, BLK], F32, tag="c" if c % 2 == 0 else "b",
                name=f"rqm{blk}_{c}")
            nc.tensor.matmul(rqm, sel, rqzT[:, c, bsl],
                             start=True, stop=True)
            nc.vector.tensor_mul(qts[:, c, bsl], qtraw[:, c, bsl], rqm)

        for g in range(TF // GRP):
            qhalf = g * GRP < TQ
            grow = ((g * GRP) % TQ) * P
            if not qhalf and g < TQ // GRP + 2:
                blk = g - TQ // GRP
                q_normalize(0, blk)
                q_normalize(1, blk)

            src2 = x2a if qhalf else x2b
            src1 = x1a if qhalf else x1b
            kq_g = kqpool.tile([P, GRP, D], BF16, tag="kq")
            nc.sync.dma_start(
                kq_g, src2[grow:grow + GRP * P, :].rearrange(
                    "(t p) d -> p t d", p=P))
            v_g = vpool.tile([P, GRP, NCH, P + 1], BF16, tag="v")
            # v loads issue on the scalar HWDGE queue: two descriptor
            # generators in parallel halves the issue serialization.
            nc.scalar.dma_start(
                v_g, src1[grow:grow + GRP * P, :].rearrange(
                    "(t p) (c q) -> p t c q", p=P, c=NCH))
            for i in range(GRP):
                t = g * GRP + i
                ekq = ekqpool.tile([P, D], BF16, tag="ekq")
                nc.scalar.activation(ekq, kq_g[:, i, :], Exp)
                for c in range(NCH):
                    nc.tensor.matmul(ctx_ps[c], ekq[:, c * P:(c + 1) * P],
                                     v_g[:, i, c, :],
                                     start=(t == 0), stop=(t == TF - 1))
                if not qhalf and g < TQ // GRP + 2 and i == GRP - 1:
                    blk = g - TQ // GRP
                    q_normalize(2, blk)
                    q_normalize(3, blk)

                if qhalf:
                    tsl = slice(t * P, (t + 1) * P)
                    qz = miscpool.tile([P, H], F32, tag="qz")
                    nc.vector.reduce_sum(
                        qz, ekq.rearrange("p (h c) -> p h c", h=H),
                        axis=mybir.AxisListType.X)
                    rqz = miscpool.tile([P, H], BF16, tag="rqz")
                    with nc.allow_low_precision("bf16 1/qz is plenty"):
                        nc.vector.reciprocal(rqz, qz)
                    tp = ps_b.tile([P, NCH, P], BF16, tag="b")
                    for c in range(NCH):
                        nc.tensor.transpose(tp[:, c, :],
                                            ekq[:, c * P:(c + 1) * P], ident)
                    nc.vector.tensor_copy(qtraw[:, :, tsl], tp)
                    # transpose 1/qz head-pair-major: [P, 2] slices ->
                    # [2, NCH, P] (pairs on partitions 0/1 for the sel mm)
                    rqt = ps_c.tile([2, NCH, P], BF16, tag="c")
                    for c in range(NCH):
                        nc.tensor.transpose(rqt[:, c, :],
                                            rqz[:, 2 * c:2 * c + 2], ident)
                    nc.vector.tensor_copy(rqzT[:, :, tsl], rqt)
            if g == TF // GRP - 1:
                nc.sync.dma_start(cb_sb, cbp[:, :])
                nc.sync.dma_start(
                    cw_sb, cwt[:, :].rearrange("(c p) o -> p c o", p=P))

        # ---- Phase 1.5: normalize context rows by Z (keys softmax),
        # transpose the block-diagonal context, fold into the conv weight:
        # Mt[c] = BD(ctx)[c]^T-normalized @ cw^T[c]  -> [128, 1024] bf16.
        for c in range(NCH):
            rz = miscpool.tile([P, 1], F32, tag="rz")
            nc.vector.reciprocal(rz, ctx_ps[c][:, P:P + 1])
            # raw block-diagonal copy; the 1/Z row scaling folds into the
            # Mt PSUM->SBUF move (per-partition scale there), keeping the
            # reciprocal off the critical chain.
            bd = bds[c]
            nc.vector.tensor_copy(bd[0:HC, 0:HC], ctx_ps[c][0:HC, 0:HC])
            nc.vector.tensor_copy(bd[HC:P, HC:P], ctx_ps[c][HC:P, HC:P])
            bdT_ps = ps_c.tile([P, P], BF16, tag="c")
            nc.tensor.transpose(bdT_ps, bd, ident)
            bdT = bdTpool.tile([P, P], BF16, tag="bdT")
            nc.vector.tensor_copy(bdT, bdT_ps)
            for half in range(2):
                osl = slice(half * (O // 2), (half + 1) * (O // 2))
                mt_ps = ps_b.tile([P, O // 2], F32, tag="b")
                nc.tensor.matmul(mt_ps, bdT, cw_sb[:, c, osl],
                                 start=True, stop=True)
                if half == 0:
                    nc.vector.tensor_scalar_mul(mt_sb[:, c, osl], mt_ps, rz)
                else:
                    nc.scalar.activation(mt_sb[:, c, osl], mt_ps, Ident,
                                         scale=rz)
            # dead transposes: keep the PE HAM activity window open while
            # the DVE/ACT chain for the next chunk drains (idle >3.4us would
            # re-throttle the clock to 1.2 GHz right before the conv burst).
            for _ in range(2):
                warm = ps_c.tile([P, P], BF16, tag="c", name="warm")
                nc.tensor.transpose(warm, ident, ident)

        # ---- Phase 2: fused conv+bias -> LayerNorm, 128 tokens per step.
        for s in range(TQ):
            tok0 = s * P
            if s in (0, 4):
                # blocks 2/3 query-normalize, emitted two blocks ahead of
                # their conv consumers (s=8/s=12): phase 1's tail stays
                # clear and the conv stream never waits on them.
                for c in range(NCH):
                    q_normalize(c, 2 + s // 4)
            # half0 cycles the 4 "a" banks; half1 alternates the "b"/"c"
            # banks -> 4 LN chains in flight, PE never stalls on PSUM WAR.
            cv0 = ps_a.tile([P, O // 2], F32, tag="a", name=f"cv0_{s}")
            cv1 = (ps_b if s % 2 == 0 else ps_c).tile(
                [P, O // 2], F32, tag="b" if s % 2 == 0 else "c",
                name=f"cv1_{s}")
            cv = [cv0, cv1]
            for half in range(2):
                osl = slice(half * (O // 2), (half + 1) * (O // 2))
                nc.tensor.matmul(cv[half], ones_row, cb_sb[:, osl],
                                 start=True, stop=False)
            for c in range(NCH):
                qsl = qts[:, c, tok0:tok0 + P]
                for half in range(2):
                    osl = slice(half * (O // 2), (half + 1) * (O // 2))
                    nc.tensor.matmul(cv[half], qsl, mt_sb[:, c, osl],
                                     start=False, stop=(c == NCH - 1))
            stats = lnpool.tile([P, 2, 6], F32, tag="stats")
            for half in range(2):
                nc.vector.bn_stats(stats[:, half, :], cv[half])
            mv = lnpool.tile([P, 2], F32, tag="mv")
            nc.vector.bn_aggr(mv, stats)
            std = lnpool.tile([P, 1], F32, tag="std")
            nc.scalar.activation(std, mv[:, 1:2], Sqrt, bias=eps_t)
            rstd = lnpool.tile([P, 1], F32, tag="rstd")
            nc.vector.reciprocal(rstd, std)
            nmr = lnpool.tile([P, 1], F32, tag="nmr")
            nc.vector.tensor_scalar(nmr, mv[:, 0:1], rstd, -1.0,
                                    Mult, Mult)
            o_sb = outpool.tile([P, O], BF16, tag="o")
            # both halves on ACT: DVE carries bn_stats + block-2/3 muls here
            for half in range(2):
                osl = slice(half * (O // 2), (half + 1) * (O // 2))
                nc.scalar.activation(o_sb[:, osl], cv[half], Ident,
                                     bias=nmr, scale=rstd)
            nc.sync.dma_start(out[tok0:tok0 + P, :], o_sb)
    return nc


_CACHE = {}


def _get_program():
    if "nc" not in _CACHE:
        nc = _build_program()
        if not nc.is_finalized():
            nc.finalize()
        _CACHE["nc"] = nc
    return _CACHE["nc"]


def _make_in_maps(x1, x2, conv_w, conv_b):
    x1e = np.ones((B, NF, NCH, P + 1), dtype=NPBF16)
    x1e[:, :, :, :P] = np.asarray(x1, dtype=np.float32).reshape(
        B, NF, NCH, P).astype(NPBF16)
    x1 = x1e.reshape(B, NF, D + NCH)
    x2 = np.ascontiguousarray(x2, dtype=np.float32).astype(NPBF16)
    cwt = np.ascontiguousarray(conv_w.T).astype(NPBF16)
    cb = np.asarray(conv_b, dtype=np.float32).reshape(1, O).astype(NPBF16)
    sel = np.zeros((2, P), dtype=NPBF16)
    sel[0, 0:HC] = 1.0
    sel[1, HC:P] = 1.0
    in_maps = []
    for core in range(N_CORES):
        b, j = core // 2, core % 2
        a_sl = slice(j * NQ, (j + 1) * NQ)
        b_sl = slice((1 - j) * NQ, (2 - j) * NQ)
        in_maps.append({
            "x1a": x1[b, a_sl], "x1b": x1[b, b_sl],
            "x2a": x2[b, a_sl], "x2b": x2[b, b_sl],
            "cwt": cwt, "cb": cb, "sel": sel,
        })
    return in_maps


def _run(x1, x2, conv_w, conv_b, trace=False):
    nc = _get_program()
    in_maps = _make_in_maps(x1, x2, conv_w, conv_b)
    res = run_bass_kernel_spmd(nc, in_maps, list(range(N_CORES)), trace=trace)
    full = np.empty((B, NF, O), dtype=np.float32)
    for core in range(N_CORES):
        b, j = core // 2, core % 2
        full[b, j * NQ:(j + 1) * NQ, :] = res.results[core]["out"].astype(
            np.float32)
    return full, res.exec_time_ns


def kernel(x1, x2, conv_w, conv_b, ln_w, ln_b):
    out, _ = _run(np.asarray(x1), np.asarray(x2),
                  np.asarray(conv_w), np.asarray(conv_b))
    ln_w = np.asarray(ln_w, dtype=np.float32)
    ln_b = np.asarray(ln_b, dtype=np.float32)
    if not (np.all(ln_w == 1.0) and np.all(ln_b == 0.0)):
        out = out * ln_w[None, None, :] + ln_b[None, None, :]
    return out


# revision 49
# speedup vs baseline: 1.0327x; 1.0327x over previous
"""Trainium2 Bass kernel for nn_Cross_Attention (linear attention + 1x1 conv + LayerNorm).

Math (per batch b):
  kq = x2[b].T (channels-first), heads h=8, 64 ch/head
  keys    = softmax(kq) over tokens N
  queries = softmax(kq) over channels-within-head
  context[h] = keys[h] @ v[h].T          (v = x1[b].T)       [64, 64]
  attended[h] = context[h].T @ queries[h]                    [64, N]
  reproj = conv_w @ concat(attended) + conv_b                [1024, N]
  out = LayerNorm_channels(reproj.T)                         [N, 1024]

Key restructure vs the straightforward pipeline: reproj = cw @ BD(ctx) @ q
is associatively refolded as (cw @ BD(ctx)) @ q, so the big GEMM consumes
the (transposed, query-normalized) exp(kq) directly and the attended
intermediate never materializes.  Mt = BD(ctx)^T @ cw^T is a tiny [512,1024]
precompute per core.

Query-softmax normalization (per head+token scalar) is applied in channel-
major layout: 1/qz head-pair rows are broadcast across partitions by a tiny
K=2 selector matmul into PSUM and fused into one elementwise multiply per
512-token block, instead of 128 tiny per-head tensor_scalars.

LayerNorm: bn_stats/bn_aggr on DVE, then the normalize itself runs on the
Scalar engine as Identity(x * rstd + (-mu*rstd)) with per-partition scale /
bias operands, reading conv PSUM directly and emitting bf16.

Sharding: 8 cores = 4 batches x 2 token-halves. Each core receives the full
batch (needed for the token-axis softmax + context), computes context
redundantly within the pair, and produces its own 2048-token half of the
output (bf16, upcast on host). No cross-core communication.

Numerics: exp/softmax inputs are bounded (randn), so the max-subtraction is
skipped. Matmuls run in bf16 with fp32 PSUM accumulation. The softmax-over-N
denominator is obtained by a ones-column matmul fused into the context
accumulation. The conv bias is injected into PSUM via a K=1 ones-row matmul
so LayerNorm stats can be taken directly from PSUM.
"""

import numpy as np
import ml_dtypes
from contextlib import ExitStack

import concourse.bass as bass
import concourse.bacc as bacc
import concourse.tile as tile
from concourse import mybir
from concourse.bass_utils import run_bass_kernel_spmd
from concourse.masks import make_identity

BF16 = mybir.dt.bfloat16
F32 = mybir.dt.float32
NPBF16 = ml_dtypes.bfloat16

P = 128          # partitions
NQ = 2048        # tokens owned by this core (query half)
NF = 4096        # full token count per batch
D = 512          # input channels
H = 8            # heads
HC = 64          # channels per head
O = 1024         # conv output channels
TQ = NQ // P     # 16 query-half token tiles
TF = NF // P     # 32 full token tiles
NCH = D // P     # 4 channel chunks (2 heads each)
LN_EPS = 1e-5
B = 4
N_CORES = 8
GRP = 4          # token tiles per input DMA

Exp = mybir.ActivationFunctionType.Exp
Sqrt = mybir.ActivationFunctionType.Sqrt
Ident = mybir.ActivationFunctionType.Identity
Mult = mybir.AluOpType.mult


def _build_program():
    # Bacc (not plain Bass): its finalize() runs move_matmul_waits_to_
    # ldweights + generate_event_semaphores, which split multi-wait
    # instructions into EventSemaphore preludes — the HW encodings allow
    # at most 1 inline wait (2 for EventSemaphore).
    nc = bacc.Bacc()
    # x1 halves arrive pre-interleaved as [NQ, 4, 129]: four 128-channel
    # chunks each followed by a literal 1.0 column (softmax-Z ones fused
    # into the context matmul's moving operand).
    x1a = nc.declare_dram_parameter("x1a", [NQ, D + NCH], BF16, isOutput=False)
    x1b = nc.declare_dram_parameter("x1b", [NQ, D + NCH], BF16, isOutput=False)
    x2a = nc.declare_dram_parameter("x2a", [NQ, D], BF16, isOutput=False)
    x2b = nc.declare_dram_parameter("x2b", [NQ, D], BF16, isOutput=False)
    cwt = nc.declare_dram_parameter("cwt", [D, O], BF16, isOutput=False)
    cbp = nc.declare_dram_parameter("cb", [1, O], BF16, isOutput=False)
    selp = nc.declare_dram_parameter("sel", [2, P], BF16, isOutput=False)
    out = nc.declare_dram_parameter("out", [NQ, O], BF16, isOutput=True)

    with tile.TileContext(nc) as tc, ExitStack() as ctx:
        singles = ctx.enter_context(tc.tile_pool(name="singles", bufs=1))
        # DMA-written pools get one buf per tile (no slot reuse): a reused
        # slot's DMA needs WAR + 2-queue WAW waits = 3 > the 2-wait limit of
        # the DMA descriptor encoding. Fresh slots -> input DMAs wait-free.
        kqpool = ctx.enter_context(tc.tile_pool(name="kq", bufs=TF // GRP))
        vpool = ctx.enter_context(tc.tile_pool(name="v", bufs=TF // GRP))
        ekqpool = ctx.enter_context(tc.tile_pool(name="ekq", bufs=6))
        bdpool = ctx.enter_context(tc.tile_pool(name="bd", bufs=NCH))
        bdTpool = ctx.enter_context(tc.tile_pool(name="bdT", bufs=NCH))
        lnpool = ctx.enter_context(tc.tile_pool(name="ln", bufs=6))
        outpool = ctx.enter_context(tc.tile_pool(name="outp", bufs=3))
        miscpool = ctx.enter_context(tc.tile_pool(name="misc", bufs=8))
        rqsbpool = ctx.enter_context(tc.tile_pool(name="rqsb", bufs=4))
        # PSUM: 8 banks of 2KB/partition total.
        # a: ctx accumulators (phase 1, 4 live) then conv halves (phase 2,
        #    disjoint lifetime) -> 4 banks.
        # b: ekq transposes (ph1) / Mt halves (ph1.5) -> 2 banks.
        # c: rqz transposes (ph1) / bdT transposes (ph1.5) -> 2 banks.
        ps_a = ctx.enter_context(tc.tile_pool(name="ps_a", bufs=4, space="PSUM"))
        ps_b = ctx.enter_context(tc.tile_pool(name="ps_b", bufs=2, space="PSUM"))
        ps_c = ctx.enter_context(tc.tile_pool(name="ps_c", bufs=2, space="PSUM"))

        # constants (weight/bias/selector loads are emitted AFTER the input
        # loop so the sync queue issues the latency-critical kq loads first)
        ident = singles.tile([P, P], BF16)
        make_identity(nc, ident)
        ones_row = singles.tile([1, P], BF16)
        nc.vector.memset(ones_row, 1.0)
        eps_t = singles.tile([P, 1], F32)
        nc.vector.memset(eps_t, LN_EPS)
        cw_sb = singles.tile([P, NCH, O], BF16)
        cb_sb = singles.tile([1, O], BF16)

        # big SBUF intermediates
        rqzT = singles.tile([2, NCH, NQ], BF16)   # 1/qz, head-pair-major
        qtraw = singles.tile([P, NCH, NQ], BF16)  # exp(kq)^T, channel-major
        qts = singles.tile([P, NCH, NQ], BF16)    # normalized queries^T
        mt_sb = singles.tile([P, NCH, O], BF16)   # Mt = BD(ctx)^T @ cw^T
        # head-pair selector (host constant): sel^T @ rqzT[:, c] broadcasts
        # head 2c to partitions 0..63 and head 2c+1 to 64..127 (PSUM),
        # replacing the per-head scalar normalization with one fused multiply.
        sel = singles.tile([2, P], BF16)
        # tiny constants via GpSimd SWDGE: keeps both HWDGE queues free for
        # the latency-critical input stream.
        nc.gpsimd.dma_start(sel, selp[:, :])
        # block-diagonal context tiles, zeroed up front (off-phase memsets)
        bds = []
        for c in range(NCH):
            bd = bdpool.tile([P, P], BF16, tag="bd", name=f"bd{c}")
            nc.vector.memset(bd, 0.0)
            bds.append(bd)

        # ---- Phase 1: exp(kq); context/Z accumulation over all 32 tiles;
        # for own-half tiles also transpose exp(kq) (sharing the stationary
        # operand with the context matmul) and produce 1/qz rows.
        # Input loads are batched 4 token-tiles per DMA: descriptor issue on
        # the sync queue costs ~620ns per DMA, so 64 single-tile loads would
        # pace the whole phase.
        ctx_ps = [ps_a.tile([P, P + 1], F32, tag="a", name=f"ctxps{i}")
                  for i in range(NCH)]
        BLK = NQ // 4

        def q_normalize(c, blk):
            # one (chunk, 512-token block) of query normalization:
            # PE broadcasts 1/qz per head pair into PSUM, DVE casts it to
            # SBUF, and the multiply itself runs on the otherwise-idle
            # GpSimd (SBUF-only operands), keeping DVE clear for the
            # context-fold chain.
            bsl = slice(blk * BLK, (blk + 1) * BLK)
            rqm = (ps_c if c % 2 == 0 else ps_b).tile(
                [P, BLK], F32, tag="c" if c % 2 == 0 else "b",
                name=f"rqm{blk}_{c}")
            nc.tensor.matmul(rqm, sel, rqzT[:, c, bsl],
                             start=True, stop=True)
            nc.vector.tensor_mul(qts[:, c, bsl], qtraw[:, c, bsl], rqm)

        for g in range(TF // GRP):
            qhalf = g * GRP < TQ
            grow = ((g * GRP) % TQ) * P
            if not qhalf:
                blk = g - TQ // GRP
                q_normalize(0, blk)
                q_normalize(1, blk)

            src2 = x2a if qhalf else x2b
            src1 = x1a if qhalf else x1b
            kq_g = kqpool.tile([P, GRP, D], BF16, tag="kq")
            nc.sync.dma_start(
                kq_g, src2[grow:grow + GRP * P, :].rearrange(
                    "(t p) d -> p t d", p=P))
            v_g = vpool.tile([P, GRP, NCH, P + 1], BF16, tag="v")
            # v loads issue on the scalar HWDGE queue: two descriptor
            # generators in parallel halves the issue serialization.
            nc.scalar.dma_start(
                v_g, src1[grow:grow + GRP * P, :].rearrange(
                    "(t p) (c q) -> p t c q", p=P, c=NCH))
            for i in range(GRP):
                t = g * GRP + i
                ekq = ekqpool.tile([P, D], BF16, tag="ekq")
                nc.scalar.activation(ekq, kq_g[:, i, :], Exp)
                for c in range(NCH):
                    nc.tensor.matmul(ctx_ps[c], ekq[:, c * P:(c + 1) * P],
                                     v_g[:, i, c, :],
                                     start=(t == 0), stop=(t == TF - 1))
                if not qhalf and i == GRP - 1:
                    blk = g - TQ // GRP
                    q_normalize(2, blk)
                    q_normalize(3, blk)

                if qhalf:
                    tsl = slice(t * P, (t + 1) * P)
                    qz = miscpool.tile([P, H], F32, tag="qz")
                    nc.vector.reduce_sum(
                        qz, ekq.rearrange("p (h c) -> p h c", h=H),
                        axis=mybir.AxisListType.X)
                    rqz = miscpool.tile([P, H], BF16, tag="rqz")
                    with nc.allow_low_precision("bf16 1/qz is plenty"):
                        nc.vector.reciprocal(rqz, qz)
                    tp = ps_b.tile([P, NCH, P], BF16, tag="b")
                    for c in range(NCH):
                        nc.tensor.transpose(tp[:, c, :],
                                            ekq[:, c * P:(c + 1) * P], ident)
                    nc.vector.tensor_copy(qtraw[:, :, tsl], tp)
                    # transpose 1/qz head-pair-major: [P, 2] slices ->
                    # [2, NCH, P] (pairs on partitions 0/1 for the sel mm)
                    rqt = ps_c.tile([2, NCH, P], BF16, tag="c")
                    for c in range(NCH):
                        nc.tensor.transpose(rqt[:, c, :],
                                            rqz[:, 2 * c:2 * c + 2], ident)
                    nc.vector.tensor_copy(rqzT[:, :, tsl], rqt)
            if g == TF // GRP - 1:
                nc.sync.dma_start(cb_sb, cbp[:, :])
                nc.sync.dma_start(
                    cw_sb, cwt[:, :].rearrange("(c p) o -> p c o", p=P))

        # ---- Phase 1.5: normalize context rows by Z (keys softmax),
        # transpose the block-diagonal context, fold into the conv weight:
        # Mt[c] = BD(ctx)[c]^T-normalized @ cw^T[c]  -> [128, 1024] bf16.
        for c in range(NCH):
            rz = miscpool.tile([P, 1], F32, tag="rz")
            nc.vector.reciprocal(rz, ctx_ps[c][:, P:P + 1])
            # raw block-diagonal copy; the 1/Z row scaling folds into the
            # Mt PSUM->SBUF move (per-partition scale there), keeping the
            # reciprocal off the critical chain.
            bd = bds[c]
            nc.vector.tensor_copy(bd[0:HC, 0:HC], ctx_ps[c][0:HC, 0:HC])
            nc.vector.tensor_copy(bd[HC:P, HC:P], ctx_ps[c][HC:P, HC:P])
            bdT_ps = ps_c.tile([P, P], BF16, tag="c")
            nc.tensor.transpose(bdT_ps, bd, ident)
            bdT = bdTpool.tile([P, P], BF16, tag="bdT")
            nc.vector.tensor_copy(bdT, bdT_ps)
            for half in range(2):
                osl = slice(half * (O // 2), (half + 1) * (O // 2))
                mt_ps = ps_b.tile([P, O // 2], F32, tag="b")
                nc.tensor.matmul(mt_ps, bdT, cw_sb[:, c, osl],
                                 start=True, stop=True)
                if half == 0:
                    nc.vector.tensor_scalar_mul(mt_sb[:, c, osl], mt_ps, rz)
                else:
                    nc.scalar.activation(mt_sb[:, c, osl], mt_ps, Ident,
                                         scale=rz)
            # dead transposes: keep the PE HAM activity window open while
            # the DVE/ACT chain for the next chunk drains (idle >3.4us would
            # re-throttle the clock to 1.2 GHz right before the conv burst).
            for _ in range(2):
                warm = ps_c.tile([P, P], BF16, tag="c", name="warm")
                nc.tensor.transpose(warm, ident, ident)

        # ---- Phase 2: fused conv+bias -> LayerNorm, 128 tokens per step.
        for s in range(TQ):
            tok0 = s * P
            # half0 cycles the 4 "a" banks; half1 alternates the "b"/"c"
            # banks -> 4 LN chains in flight, PE never stalls on PSUM WAR.
            cv0 = ps_a.tile([P, O // 2], F32, tag="a", name=f"cv0_{s}")
            cv1 = (ps_b if s % 2 == 0 else ps_c).tile(
                [P, O // 2], F32, tag="b" if s % 2 == 0 else "c",
                name=f"cv1_{s}")
            cv = [cv0, cv1]
            for half in range(2):
                osl = slice(half * (O // 2), (half + 1) * (O // 2))
                nc.tensor.matmul(cv[half], ones_row, cb_sb[:, osl],
                                 start=True, stop=False)
            for c in range(NCH):
                qsl = qts[:, c, tok0:tok0 + P]
                for half in range(2):
                    osl = slice(half * (O // 2), (half + 1) * (O // 2))
                    nc.tensor.matmul(cv[half], qsl, mt_sb[:, c, osl],
                                     start=False, stop=(c == NCH - 1))
            stats = lnpool.tile([P, 2, 6], F32, tag="stats")
            for half in range(2):
                nc.vector.bn_stats(stats[:, half, :], cv[half])
            mv = lnpool.tile([P, 2], F32, tag="mv")
            nc.vector.bn_aggr(mv, stats)
            std = lnpool.tile([P, 1], F32, tag="std")
            nc.scalar.activation(std, mv[:, 1:2], Sqrt, bias=eps_t)
            rstd = lnpool.tile([P, 1], F32, tag="rstd")
            nc.vector.reciprocal(rstd, std)
            nmr = lnpool.tile([P, 1], F32, tag="nmr")
            nc.vector.tensor_scalar(nmr, mv[:, 0:1], rstd, -1.0,
                                    Mult, Mult)
            o_sb = outpool.tile([P, O], BF16, tag="o")
            # normalize halves in parallel on the two PSUM-capable engines
            nc.scalar.activation(o_sb[:, 0:O // 2], cv[0], Ident,
                                 bias=nmr, scale=rstd)
            nc.vector.tensor_scalar(o_sb[:, O // 2:O], cv[1], mv[:, 0:1],
                                    rstd, mybir.AluOpType.subtract, Mult)
            nc.sync.dma_start(out[tok0:tok0 + P, :], o_sb)
    return nc


_CACHE = {}


def _get_program():
    if "nc" not in _CACHE:
        nc = _build_program()
        if not nc.is_finalized():
            nc.finalize()
        _CACHE["nc"] = nc
    return _CACHE["nc"]


def _make_in_maps(x1, x2, conv_w, conv_b):
    x1e = np.ones((B, NF, NCH, P + 1), dtype=NPBF16)
    x1e[:, :, :, :P] = np.asarray(x1, dtype=np.float32).reshape(
        B, NF, NCH, P).astype(NPBF16)
    x1 = x1e.reshape(B, NF, D + NCH)
    x2 = np.ascontiguousarray(x2, dtype=np.float32).astype(NPBF16)
    cwt = np.ascontiguousarray(conv_w.T).astype(NPBF16)
    cb = np.asarray(conv_b, dtype=np.float32).reshape(1, O).astype(NPBF16)
    sel = np.zeros((2, P), dtype=NPBF16)
    sel[0, 0:HC] = 1.0
    sel[1, HC:P] = 1.0
    in_maps = []
    for core in range(N_CORES):
        b, j = core // 2, core % 2
        a_sl = slice(j * NQ, (j + 1) * NQ)
        b_sl = slice((1 - j) * NQ, (2 - j) * NQ)
        in_maps.append({
            "x1a": x1[b, a_sl], "x1b": x1[b, b_sl],
            "x2a": x2[b, a_sl], "x2b": x2[b, b_sl],
            "cwt": cwt, "cb": cb, "sel": sel,
        })
    return in_maps


def _run(x1, x2, conv_w, conv_b, trace=False):
    nc = _get_program()
    in_maps = _make_in_maps(x1, x2, conv_w, conv_b)
    res = run_bass_kernel_spmd(nc, in_maps, list(range(N_CORES)), trace=trace)
    full = np.empty((B, NF, O), dtype=np.float32)
    for core in range(N_CORES):
        b, j = core // 2, core % 2
        full[b, j * NQ:(j + 1) * NQ, :] = res.results[core]["out"].astype(
            np.float32)
    return full, res.exec_time_ns


def kernel(x1, x2, conv_w, conv_b, ln_w, ln_b):
    out, _ = _run(np.asarray(x1), np.asarray(x2),
                  np.asarray(conv_w), np.asarray(conv_b))
    ln_w = np.asarray(ln_w, dtype=np.float32)
    ln_b = np.asarray(ln_b, dtype=np.float32)
    if not (np.all(ln_w == 1.0) and np.all(ln_b == 0.0)):
        out = out * ln_w[None, None, :] + ln_b[None, None, :]
    return out
